# revision 1
# baseline (speedup 1.0000x reference)
"""Causal self-attention with LoRA (q,v) — Trainium2 Bass kernel, 8 cores.

Sharding: data-parallel over batch (B=2), tensor-parallel over heads
(16 heads -> 4 per core).  Core c handles batch c//4, heads 4*(c%4)..+4.
Each core computes its 256-dim q/k/v projection slice from the full
hidden states and its heads' full 2048x2048 causal attention locally.
No collectives; host does the (layout-only) scatter/gather.

All matmuls run as float32r (TF32-like, full PE rate at N>=256); every
tensor feeding a matmul is produced as f32r (gpsimd casting DMAs for
loads, f32r-dtype outputs for compute), which the BIR verifier requires.
Attention is computed in transposed orientation (scores sT[j, i]) so no
on-chip transposes are needed; the softmax denominator rides along the
PV matmul as a 65th lhsT column (augmented-V), which also folds the
additive attention mask in exactly (em = exp(mask) scaling of V rows).
Causal masking is a multiplicative staircase applied after exp; the two
most-diagonal 128-key chunks per 512-query block are cropped to their
upper 256 columns (laid out so concurrently issued row-packed matmul
pairs never write the same PSUM bank, which crashes the HW otherwise).
Scores for the two heads of a pair run concurrently via K=64 row
packing (tile_position from base partitions 0/64).  HW exec time:
~210 us/core, rel err vs fp32 reference ~2.2e-4.
"""

import numpy as np

B, T, DM, H = 2, 2048, 1024, 16
HD = 64          # head dim
R = 8            # LoRA rank
NCORES = 8
GPB = 4          # head-groups (cores) per batch
HPC = 4          # heads per core
DPC = HPC * HD   # 256 output dims per core
LORA_SCALE = 2.0  # lora_alpha / r
SM_SCALE = HD ** -0.5  # 0.125

KC = DM // 128   # 8 contraction chunks
MC = DPC // 128  # 2 output-dim chunks (head pairs)
NB = T // 512    # 4 t-blocks for q/k projections
TCH = T // 128   # 16 t-chunks (key chunks)
IBN = T // 512   # 4 query i-blocks (512 wide)
GW = 1024        # score psum group width (2 banks = one (h,h') jb pair)

_CACHE = {}
import os
USE_APPROX_RECIP = os.environ.get("APPROX_RECIP", "") != ""  # custom-DVE op broken on this runtime
WK_GPSIMD_DMA = os.environ.get("WK_GPSIMD_DMA", "") != ""
NO_CROP = os.environ.get("NO_CROP", "") != ""  # diagonal-chunk crop (bank-safe layout)
NO_MASK3D = os.environ.get("NO_MASK3D", "") != ""
SC3 = os.environ.get("NO_SC3", "") == ""   # scores psum bufs=3, po bufs=1
WARMUP = int(os.environ.get("WARMUP", "0"))  # dummy warmup matmuls


def _build_program():
    from contextlib import ExitStack

    import concourse.bass as bass
    import concourse.tile as tile
    from concourse import bacc, mybir

    f32 = mybir.dt.float32
    f32r = mybir.dt.float32r
    EXP = mybir.ActivationFunctionType.Exp
    COPY = mybir.ActivationFunctionType.Copy
    ts = bass.ts

    nc = bacc.Bacc(
        "TRN2",
        target_bir_lowering=False,
        debug=False,
        enable_asserts=True,
        num_devices=NCORES,
    )

    xT = nc.dram_tensor("xT", [DM, T], f32, kind="ExternalInput").ap()
    wqT = nc.dram_tensor("wqT", [DM, DPC], f32, kind="ExternalInput").ap()
    wkT = nc.dram_tensor("wkT", [DM, DPC], f32, kind="ExternalInput").ap()
    wvT = nc.dram_tensor("wvT", [DM, DPC], f32, kind="ExternalInput").ap()
    aq = nc.dram_tensor("aq", [R, DM], f32, kind="ExternalInput").ap()
    av = nc.dram_tensor("av", [R, DM], f32, kind="ExternalInput").ap()
    bqT = nc.dram_tensor("bqT", [R, DPC], f32, kind="ExternalInput").ap()
    bvT = nc.dram_tensor("bvT", [R, DPC], f32, kind="ExternalInput").ap()
    biasq = nc.dram_tensor("biasq", [DPC], f32, kind="ExternalInput").ap()
    biask = nc.dram_tensor("biask", [DPC], f32, kind="ExternalInput").ap()
    biasv = nc.dram_tensor("biasv", [DPC], f32, kind="ExternalInput").ap()
    amask = nc.dram_tensor("amask", [T], f32, kind="ExternalInput").ap()
    outT = nc.dram_tensor("outT", [DPC, T], f32, kind="ExternalOutput").ap()

    with tile.TileContext(nc) as tc, ExitStack() as ctx:
        const = ctx.enter_context(tc.tile_pool(name="const", bufs=1))
        xpool = ctx.enter_context(tc.tile_pool(name="x", bufs=1))
        wpool = ctx.enter_context(tc.tile_pool(name="w", bufs=1))
        wtmp = ctx.enter_context(tc.tile_pool(name="wtmp", bufs=3))
        qkpool = ctx.enter_context(tc.tile_pool(name="qk", bufs=1))
        vpool = ctx.enter_context(tc.tile_pool(name="v", bufs=1))
        ppool = ctx.enter_context(tc.tile_pool(name="pT", bufs=4))
        opool = ctx.enter_context(tc.tile_pool(name="osb", bufs=4))
        psum = ctx.enter_context(tc.tile_pool(name="psum", bufs=3 if SC3 else 2, space="PSUM"))
        popool = ctx.enter_context(tc.tile_pool(name="po", bufs=1 if SC3 else 2, space="PSUM"))

        def stair_slice(d, w):
            # full-width chunk (w=512): mask[p, f] = f >= p + 128 d
            # cropped chunk  (w=256, f' = f-256): f' >= p + 128 d - 256
            start = 384 - 128 * d if w == 512 else 640 - 128 * d
            return stair[:, start : start + w]

        ones_f = const.tile([1, 128], f32, tag="ones_f")
        nc.vector.memset(ones_f[:], 1.0)
        ones_1xP = const.tile([1, 128], f32r, tag="ones")
        nc.vector.tensor_copy(ones_1xP[:], ones_f[:])

        # em[p, jb] = exp(amask[128*jb + p])
        em_raw = const.tile([128, TCH], f32, tag="em_raw")
        nc.sync.dma_start(em_raw[:], amask.rearrange("(c p) -> p c", p=128))
        em = const.tile([128, TCH], f32, tag="em")
        nc.scalar.activation(em[:], em_raw[:], EXP)

        bias_q = []
        bias_k = []
        for mc in range(MC):
            tq = const.tile([128, 1], f32, tag=f"bq{mc}")
            nc.sync.dma_start(tq[:], biasq[ts(mc, 128)].unsqueeze(1))
            bias_q.append(tq)
            tk = const.tile([128, 1], f32, tag=f"bk{mc}")
            nc.sync.dma_start(tk[:], biask[ts(mc, 128)].unsqueeze(1))
            bias_k.append(tk)
        bv_row = const.tile([1, DPC], f32r, tag="bvrow")
        nc.gpsimd.dma_start(bv_row[:], biasv.unsqueeze(0))

        # LoRA operands (gpsimd queue, ahead of anything big).
        # bqT/bvT arrive pre-scaled by 2 (lora_alpha/r) from the host.
        aq_sb = const.tile([R, DM], f32r, tag="aq")
        nc.gpsimd.dma_start(aq_sb[:], aq)
        av_sb = const.tile([R, DM], f32r, tag="av")
        nc.gpsimd.dma_start(av_sb[:], av)
        bqT2 = const.tile([R, DPC], f32r, tag="bqT2")
        nc.gpsimd.dma_start(bqT2[:], bqT)
        bvT2 = const.tile([R, DPC], f32r, tag="bvT2")
        nc.gpsimd.dma_start(bvT2[:], bvT)

        # ---------------- x.T: casting DMAs, first in the gpsimd queue.
        # Loaded in column quarters so the first projection n-block can
        # start after ~2MB has landed.
        x_sb = []
        for kc in range(KC):
            t = xpool.tile([128, T], f32r, tag=f"x{kc}")
            x_sb.append(t)
        for q in range(4):
            for kc in range(KC):
                nc.gpsimd.dma_start(
                    x_sb[kc][:, ts(q, T // 4)],
                    xT[ts(kc, 128), ts(q, T // 4)],
                )

        # Causal staircase (multiplicative, applied after exp):
        # stair[p, m] = 1.0 if m >= p + 384 else 0.0 ; shape [128, 896].
        stair = const.tile([128, 896], f32, tag="stair")
        nc.gpsimd.memset(stair[:], 1.0)
        nc.gpsimd.affine_select(
            out=stair[:],
            in_=stair[:],
            compare_op=mybir.AluOpType.is_ge,
            fill=0.0,
            base=-384,
            pattern=[[1, 896]],
            channel_multiplier=-1,
        )

        # ---------------- weights with LoRA fold for q, v ---------------
        def load_folded(w_dram, a_sb, bT2_sb, name):
            """W'.T chunks = W.T + A.T @ (2 B.T), as 8 [128, DPC] tiles."""
            out_tiles = []
            for kc in range(KC):
                raw = wtmp.tile([128, DPC], f32, tag="wtmp")
                nc.sync.dma_start(raw[:], w_dram[ts(kc, 128), :])
                dps = psum.tile([128, DPC], f32, tag="sc")
                nc.tensor.matmul(
                    dps[:],
                    a_sb[:, ts(kc, 128)],
                    bT2_sb[:],
                    start=True,
                    stop=True,
                )
                wt = wpool.tile([128, DPC], f32r, tag=f"{name}{kc}")
                nc.vector.tensor_add(wt[:], raw[:], dps[:])
                out_tiles.append(wt)
            return out_tiles

        wq_sb = load_folded(wqT, aq_sb, bqT2, "wq")

        # wk: HWDGE f32 load into staging + tiny gpsimd rounding copy
        wk_sb = []
        for kc in range(KC):
            t = wpool.tile([128, DPC], f32r, tag=f"wk{kc}")
            if WK_GPSIMD_DMA:
                nc.gpsimd.dma_start(t[:], wkT[ts(kc, 128), :])
            else:
                stg = wtmp.tile([128, DPC], f32, tag="wkstg")
                nc.sync.dma_start(stg[:], wkT[ts(kc, 128), :])
                nc.gpsimd.tensor_copy(t[:], stg[:])
            wk_sb.append(t)

        wv_sb = load_folded(wvT, av_sb, bvT2, "wv")

        if WARMUP:
            wu_sink = const.tile([1, 1], f32, tag="wu_sink")
            for wi in range(WARMUP):
                wps = psum.tile([128, 512], f32, tag="sc")
                nc.tensor.matmul(
                    wps[:],
                    aq_sb[:, ts(wi % 8, 128)],
                    av_sb[:, ts(wi % 2, 512)],
                    start=True,
                    stop=True,
                )
                if wi == WARMUP - 1:
                    nc.vector.tensor_copy(wu_sink[:], wps[0:1, 0:1])

        # ---------------- projections ----------------
        # qT/kT: [d, t] with d on partitions; tile mc holds head pair
        # (2mc, 2mc+1): partitions 0-63 = head 2mc, 64-127 = head 2mc+1.
        qT_sb = [qkpool.tile([128, T], f32r, tag=f"qT{mc}", name=f"qT{mc}") for mc in range(MC)]
        kT_sb = [qkpool.tile([128, T], f32r, tag=f"kT{mc}", name=f"kT{mc}") for mc in range(MC)]

        def project_qk(w_tiles, dst, bias, mc):
            for nb in range(NB):
                ps = psum.tile([128, 512], f32, tag="sc")
                for kc in range(KC):
                    nc.tensor.matmul(
                        ps[:],
                        w_tiles[kc][:, ts(mc, 128)],
                        x_sb[kc][:, ts(nb, 512)],
                        start=(kc == 0),
                        stop=(kc == KC - 1),
                    )
                nc.vector.tensor_add(
                    dst[:, ts(nb, 512)],
                    ps[:],
                    bias[:].to_broadcast((128, 512)),
                )

        # v in natural [t, d] orientation, em-scaled, with the denominator
        # (em) column appended per head: [128, 4*65].
        v2_sb = [vpool.tile([128, HPC * (HD + 1)], f32r, tag=f"v2{j}", name=f"v2{j}") for j in range(TCH)]

        def project_v(jbs):
            for jb in jbs:
                ps = psum.tile([128, DPC], f32, tag="sc")
                for kc in range(KC):
                    nc.tensor.matmul(
                        ps[:],
                        x_sb[kc][:, ts(jb, 128)],
                        wv_sb[kc][:],
                        start=(kc == 0),
                        stop=False,
                    )
                nc.tensor.matmul(  # + ones(t) x bias_v
                    ps[:],
                    ones_1xP[:],
                    bv_row[:],
                    start=False,
                    stop=True,
                )
                v2 = v2_sb[jb]
                em_col = em[:, jb : jb + 1]
                for hl in range(HPC):
                    nc.vector.tensor_mul(
                        v2[:, hl * (HD + 1) : hl * (HD + 1) + HD],
                        ps[:, ts(hl, HD)],
                        em_col.to_broadcast((128, HD)),
                    )
                nc.vector.tensor_copy(
                    v2[:, HD : HPC * (HD + 1) : HD + 1],
                    em_col.to_broadcast((128, HPC)),
                )

        # ---------------- attention for one head pair ----------------
        def attention_ib(pr, ib):
            qT, kT = qT_sb[pr], kT_sb[pr]
            if True:
                nch = 4 * ib + 4  # causal key chunks per head
                # chunk stream: (hl, jb, qoff, width, d); d = diagonal
                # offset; d in {2,3} chunks cropped to the upper 256 cols.
                # full-width (512) chunks in groups of <=3; the four
                # cropped diagonal (256) chunks in their own group so every
                # matmul dst stays bank-aligned (a 512-wide psum write at a
                # non-512-aligned column would cross a bank boundary).
                fulls, crops = [], []
                for jb in range(nch):
                    d = jb - 4 * ib
                    if d >= 2 and not NO_CROP:
                        for hl in (0, 1):
                            crops.append((hl, jb, 256, 256, d))
                    else:
                        for hl in (0, 1):
                            fulls.append((hl, jb, 0, 512, d))
                # groups: list of (chunk, col_off). Fulls: one (h, h') jb
                # pair per [128, 1024] group, cols 0/512. Crops: all four
                # 256-wide diagonal chunks in one group, laid out so
                # concurrently-issued (h, h') row-packed pairs never write
                # the same psum bank: h d2 -> 0, h' d2 -> 512, h d3 -> 256,
                # h' d3 -> 768.
                groups = []
                for i in range(0, len(fulls), 2):
                    groups.append([(c, j * 512) for j, c in enumerate(fulls[i : i + 2])])
                if crops:
                    cg = [
                        (crops[0], 0),
                        (crops[1], 512),
                        (crops[2], 256),
                        (crops[3], 768),
                    ]
                    groups.append(cg)

                po = [
                    popool.tile([65, 512], f32, tag=f"po{hl}", name=f"po{pr}_{ib}_{hl}")
                    for hl in (0, 1)
                ]
                for g in groups:
                    width = sum(c[3] for c, _ in g)
                    ps = psum.tile([128, width], f32, tag="sc")
                    offs = [off for _, off in g]
                    for (hl, jb, qo, w, d), off in g:
                        nc.tensor.matmul(
                            ps[:, off : off + w],
                            kT[ts(hl, 64), ts(jb, 128)],
                            qT[ts(hl, 64), ib * 512 + qo : ib * 512 + qo + w],
                            start=True,
                            stop=True,
                        )
                    pT = ppool.tile([128, width], f32r, tag="pT")
                    nc.scalar.activation(pT[:], ps[:], EXP, scale=SM_SCALE)
                    # causal staircase on partial chunks; merge the
                    # (h, h') twin segments into one 3-D op when adjacent
                    i = 0
                    while i < len(g):
                        (hl, jb, qo, w, d), off_i = g[i]
                        if d < 0:
                            i += 1
                            continue
                        msk = stair_slice(d, w)
                        twin = (
                            not NO_MASK3D
                            and i + 1 < len(g)
                            and g[i + 1][0][1] == jb
                            and g[i + 1][0][3] == w
                            and g[i + 1][1] == off_i + w
                        )
                        if twin:
                            seg = pT[:, off_i : off_i + 2 * w].rearrange(
                                "p (two n) -> p two n", two=2
                            )
                            nc.vector.tensor_mul(
                                seg,
                                seg,
                                msk.unsqueeze(1).broadcast_to((128, 2, w)),
                            )
                            i += 2
                        else:
                            nc.vector.tensor_mul(
                                pT[:, off_i : off_i + w],
                                pT[:, off_i : off_i + w],
                                msk,
                            )
                            i += 1
                    # PV: outT[d, i] accumulation per head; denominator
                    # column (em) rides along as lhsT column 64.
                    for (hl, jb, qo, w, d), off in g:
                        nc.tensor.matmul(
                            po[hl][:, qo : qo + w],
                            v2_sb[jb][:, (2 * pr + hl) * (HD + 1) : (2 * pr + hl + 1) * (HD + 1)],
                            pT[:, off : off + w],
                            start=(jb == 0),
                            stop=(jb == nch - 1),
                        )
                # normalize: out[:64] / denom (row 64), per column
                for hl in (0, 1):
                    sbp = opool.tile([65, 512], f32, tag="sbp")
                    nc.vector.tensor_copy(sbp[:], po[hl][:])
                    # reshape the denominator row onto 128 partitions so the
                    # (slow, free-size-bound) DVE reciprocal runs on [128, 4]
                    dn = opool.tile([128, 4], f32, tag="dn")
                    nc.gpsimd.dma_start(
                        dn[:], sbp[64:65, :].rearrange("o (p c) -> o p c", p=128)
                    )
                    dnr = opool.tile([128, 4], f32, tag="dnr")
                    nc.vector.reciprocal(dnr[:], dn[:])
                    rc = opool.tile([1, 512], f32, tag="rc")
                    nc.sync.dma_start(
                        rc[:].rearrange("o (p c) -> o p c", p=128), dnr[:]
                    )
                    rb = opool.tile([64, 512], f32, tag="rb")
                    nc.gpsimd.partition_broadcast(rb[:], rc[:])
                    oT = opool.tile([64, 512], f32, tag="oT")
                    nc.vector.tensor_mul(oT[:], sbp[0:64, :], rb[:])
                    nc.sync.dma_start(
                        outT[(2 * pr + hl) * HD : (2 * pr + hl + 1) * HD, ts(ib, 512)],
                        oT[:],
                    )

        # emission order chosen for overlap: pair-0 q/k first, then v,
        # then pair-0 attention with pair-1 projections interleaved as
        # PE filler work (keeps PE dense/warm while ACT exps pair 0).
        project_qk(wq_sb, qT_sb[0], bias_q[0], 0)
        project_qk(wk_sb, kT_sb[0], bias_k[0], 0)
        project_v(range(0, 4))
        attention_ib(0, 0)
        project_v(range(4, 8))
        attention_ib(0, 1)
        project_qk(wq_sb, qT_sb[1], bias_q[1], 1)
        project_v(range(8, 12))
        attention_ib(0, 2)
        project_qk(wk_sb, kT_sb[1], bias_k[1], 1)
        project_v(range(12, 16))
        attention_ib(0, 3)
        for ib in range(IBN):
            attention_ib(1, ib)

    nc.compile()
    return nc


def _shard_inputs(inputs):
    """Full inputs -> per-core input maps (host-side layout work only)."""
    hs = np.asarray(inputs["hidden_states"], dtype=np.float32)
    am = np.asarray(inputs["attention_mask"], dtype=np.float32)
    Wq = np.asarray(inputs["Wq"], dtype=np.float32)
    Wk = np.asarray(inputs["Wk"], dtype=np.float32)
    Wv = np.asarray(inputs["Wv"], dtype=np.float32)
    bq = np.asarray(inputs["bq"], dtype=np.float32)
    bk = np.asarray(inputs["bk"], dtype=np.float32)
    bv = np.asarray(inputs["bv"], dtype=np.float32)
    Aq = np.asarray(inputs["Aq"], dtype=np.float32)
    Bq = np.asarray(inputs["Bq"], dtype=np.float32)
    Av = np.asarray(inputs["Av"], dtype=np.float32)
    Bv = np.asarray(inputs["Bv"], dtype=np.float32)

    c = np.ascontiguousarray
    xTs = [c(hs[b].T) for b in range(B)]
    in_maps = []
    for core in range(NCORES):
        b, g = core // GPB, core % GPB
        sl = slice(g * DPC, (g + 1) * DPC)
        in_maps.append(
            {
                "xT": xTs[b],
                "wqT": c(Wq[sl].T),
                "wkT": c(Wk[sl].T),
                "wvT": c(Wv[sl].T),
                "aq": c(Aq),
                "av": c(Av),
                "bqT": c(LORA_SCALE * Bq[sl].T),
                "bvT": c(LORA_SCALE * Bv[sl].T),
                "biasq": c(bq[sl]),
                "biask": c(bk[sl]),
                "biasv": c(bv[sl]),
                "amask": c(am[b, 0, 0, :]),
            }
        )
    return in_maps


def _run(inputs, trace=False):
    from concourse.bass_utils import run_bass_kernel_spmd

    if "nc" not in _CACHE:
        _CACHE["nc"] = _build_program()
    nc = _CACHE["nc"]
    in_maps = _shard_inputs(inputs)
    res = run_bass_kernel_spmd(nc, in_maps, list(range(NCORES)), trace=trace)
    out = np.empty((B, T, DM), dtype=np.float32)
    for core in range(NCORES):
        b, g = core // GPB, core % GPB
        out[b, :, g * DPC : (g + 1) * DPC] = res.results[core]["outT"].T
    return out, res


def kernel(**inputs) -> np.ndarray:
    out, _ = _run(inputs, trace=False)
    return out



# revision 4
# speedup vs baseline: 1.3256x; 1.3256x over previous
"""Causal self-attention with LoRA (q,v) — Trainium2 Bass kernel, 8 cores.

Sharding: data-parallel over batch (B=2), tensor-parallel over heads
(16 heads -> 4 per core).  Core c handles batch c//4, heads 4*(c%4)..+4.
Each core computes its 256-dim q/k/v projection slice from the full
hidden states and its heads' full 2048x2048 causal attention locally.
No collectives; host does the (layout-only) scatter/gather + fp16 casts.

The whole datapath runs in fp16 with fp32 PSUM accumulation (measured
rel err vs the fp32 reference ~5e-4).  fp16 matters a lot on TRN2: f32r
matmuls cannot use standalone LDWEIGHTS (walrus bug) so every f32r MM
pays an inline ~214ns weight load, and FP32_HIGH disables fast weight
load entirely; fp16 streams at 1 col/cycle with LDW hidden by the PE
reorder window.  fp16 inputs are cast on host, so every DMA is a plain
HWDGE transfer (the old f32r version burned ~40us of gpsimd software
casting-DMA time at startup).

Attention is computed in transposed orientation (scores sT[j, i]) so no
on-chip transposes are needed.  The additive attention mask is folded
into the exp as a per-partition (=per-key) bias on the ACT instruction;
the softmax denominator rides along the PV matmul as a 65th all-ones
lhsT column.  Causal masking is a multiplicative staircase applied
after exp; the two most-diagonal 128-key chunks per 512-query block are
cropped to their upper 256 columns (laid out so concurrently issued
row-packed matmul pairs never write the same PSUM bank).  Scores for
the two heads of a pair run concurrently via K=64 row packing.
"""

import numpy as np

B, T, DM, H = 2, 2048, 1024, 16
HD = 64          # head dim
R = 8            # LoRA rank
NCORES = 8
GPB = 4          # head-groups (cores) per batch
HPC = 4          # heads per core
DPC = HPC * HD   # 256 output dims per core
LORA_SCALE = 2.0  # lora_alpha / r
SM_SCALE = HD ** -0.5  # 0.125

KC = DM // 128   # 8 contraction chunks
MC = DPC // 128  # 2 output-dim chunks (head pairs)
NBW = 512        # q/k projection block width (ISA moving-operand max for fp16)
NB = T // NBW    # 4 t-blocks for q/k projections
TCH = T // 128   # 16 t-chunks (key chunks)
IBN = T // 512   # 4 query i-blocks (512 wide)

_CACHE = {}


def _build_program():
    from contextlib import ExitStack

    import concourse.bass as bass
    import concourse.tile as tile
    from concourse import bacc, mybir

    f32 = mybir.dt.float32
    f16 = mybir.dt.float16
    EXP = mybir.ActivationFunctionType.Exp
    ts = bass.ts

    nc = bacc.Bacc(
        "TRN2",
        target_bir_lowering=False,
        debug=False,
        enable_asserts=True,
        num_devices=NCORES,
    )

    xT = nc.dram_tensor("xT", [DM, T], f16, kind="ExternalInput").ap()
    wqT = nc.dram_tensor("wqT", [DM, DPC], f16, kind="ExternalInput").ap()
    wkT = nc.dram_tensor("wkT", [DM, DPC], f16, kind="ExternalInput").ap()
    wvT = nc.dram_tensor("wvT", [DM, DPC], f16, kind="ExternalInput").ap()
    aq = nc.dram_tensor("aq", [R, DM], f16, kind="ExternalInput").ap()
    av = nc.dram_tensor("av", [R, DM], f16, kind="ExternalInput").ap()
    bqT = nc.dram_tensor("bqT", [R, DPC], f16, kind="ExternalInput").ap()
    bvT = nc.dram_tensor("bvT", [R, DPC], f16, kind="ExternalInput").ap()
    biasq = nc.dram_tensor("biasq", [DPC], f32, kind="ExternalInput").ap()
    biask = nc.dram_tensor("biask", [DPC], f32, kind="ExternalInput").ap()
    biasv = nc.dram_tensor("biasv", [DPC], f16, kind="ExternalInput").ap()
    amask = nc.dram_tensor("amask", [T], f32, kind="ExternalInput").ap()
    outT = nc.dram_tensor("outT", [DPC, T], f32, kind="ExternalOutput").ap()

    with tile.TileContext(nc) as tc, ExitStack() as ctx:
        const = ctx.enter_context(tc.tile_pool(name="const", bufs=1))
        xpool = ctx.enter_context(tc.tile_pool(name="x", bufs=1))
        wpool = ctx.enter_context(tc.tile_pool(name="w", bufs=1))
        wtmp = ctx.enter_context(tc.tile_pool(name="wtmp", bufs=3))
        qkpool = ctx.enter_context(tc.tile_pool(name="qk", bufs=1))
        vpool = ctx.enter_context(tc.tile_pool(name="v", bufs=1))
        ppool = ctx.enter_context(tc.tile_pool(name="pT", bufs=4))
        opool = ctx.enter_context(tc.tile_pool(name="osb", bufs=4))
        psum = ctx.enter_context(tc.tile_pool(name="psum", bufs=3, space="PSUM"))
        popool = ctx.enter_context(tc.tile_pool(name="po", bufs=1, space="PSUM"))

        def stair_slice(d, w):
            # full-width chunk (w=512): mask[p, f] = f >= p + 128 d
            # cropped chunk  (w=256, f' = f-256): f' >= p + 128 d - 256
            start = 384 - 128 * d if w == 512 else 640 - 128 * d
            return stair[:, start : start + w]

        ones_1xP = const.tile([1, 128], f16, tag="ones")
        nc.vector.memset(ones_1xP[:], 1.0)

        # am_sb[p, jb] = amask[128*jb + p]  (exp bias per key chunk)
        am_sb = const.tile([128, TCH], f32, tag="am")
        nc.sync.dma_start(am_sb[:], amask.rearrange("(c p) -> p c", p=128))

        bias_q = []
        bias_k = []
        for mc in range(MC):
            tq = const.tile([128, 1], f32, tag=f"bq{mc}")
            nc.sync.dma_start(tq[:], biasq[ts(mc, 128)].unsqueeze(1))
            bias_q.append(tq)
            tk = const.tile([128, 1], f32, tag=f"bk{mc}")
            nc.sync.dma_start(tk[:], biask[ts(mc, 128)].unsqueeze(1))
            bias_k.append(tk)
        bv_row = const.tile([1, DPC], f16, tag="bvrow")
        nc.sync.dma_start(bv_row[:], biasv.unsqueeze(0))

        # LoRA operands (scalar HWDGE queue, ahead of x).
        # bqT/bvT arrive pre-scaled by 2 (lora_alpha/r) from the host.
        aq_sb = const.tile([R, DM], f16, tag="aq")
        nc.scalar.dma_start(aq_sb[:], aq)
        av_sb = const.tile([R, DM], f16, tag="av")
        nc.scalar.dma_start(av_sb[:], av)
        bqT2 = const.tile([R, DPC], f16, tag="bqT2")
        nc.scalar.dma_start(bqT2[:], bqT)
        bvT2 = const.tile([R, DPC], f16, tag="bvT2")
        nc.scalar.dma_start(bvT2[:], bvT)

        # ---------------- x.T: fp16 HWDGE loads on the scalar queue.
        # Loaded in column quarters so the first projection block can
        # start after ~2MB has landed.
        x_sb = []
        for kc in range(KC):
            t = xpool.tile([128, T], f16, tag=f"x{kc}")
            x_sb.append(t)
        for q in range(4):
            for kc in range(KC):
                nc.scalar.dma_start(
                    x_sb[kc][:, ts(q, T // 4)],
                    xT[ts(kc, 128), ts(q, T // 4)],
                )

        # Causal staircase (multiplicative, applied after exp):
        # stair[p, m] = 1.0 if m >= p + 384 else 0.0 ; shape [128, 896].
        stair = const.tile([128, 896], f16, tag="stair")
        nc.gpsimd.memset(stair[:], 1.0)
        nc.gpsimd.affine_select(
            out=stair[:],
            in_=stair[:],
            compare_op=mybir.AluOpType.is_ge,
            fill=0.0,
            base=-384,
            pattern=[[1, 896]],
            channel_multiplier=-1,
        )

        # ---------------- weights with LoRA fold for q, v ---------------
        def load_folded(w_dram, a_sb, bT2_sb, name):
            """W'.T chunks = W.T + A.T @ (2 B.T), as 8 [128, DPC] tiles."""
            out_tiles = []
            for kc in range(KC):
                raw = wtmp.tile([128, DPC], f16, tag="wtmp")
                nc.sync.dma_start(raw[:], w_dram[ts(kc, 128), :])
                dps = psum.tile([128, DPC], f32, tag="sc")
                nc.tensor.matmul(
                    dps[:],
                    a_sb[:, ts(kc, 128)],
                    bT2_sb[:],
                    start=True,
                    stop=True,
                )
                wt = wpool.tile([128, DPC], f16, tag=f"{name}{kc}")
                nc.vector.tensor_add(wt[:], raw[:], dps[:])
                out_tiles.append(wt)
            return out_tiles

        wq_sb = load_folded(wqT, aq_sb, bqT2, "wq")

        wk_sb = []
        for kc in range(KC):
            t = wpool.tile([128, DPC], f16, tag=f"wk{kc}")
            nc.sync.dma_start(t[:], wkT[ts(kc, 128), :])
            wk_sb.append(t)

        wv_sb = load_folded(wvT, av_sb, bvT2, "wv")

        # ---------------- projections ----------------
        # qT/kT: [d, t] with d on partitions; tile mc holds head pair
        # (2mc, 2mc+1): partitions 0-63 = head 2mc, 64-127 = head 2mc+1.
        qT_sb = [qkpool.tile([128, T], f16, tag=f"qT{mc}", name=f"qT{mc}") for mc in range(MC)]
        kT_sb = [qkpool.tile([128, T], f16, tag=f"kT{mc}", name=f"kT{mc}") for mc in range(MC)]

        def project_qk(w_tiles, dst, bias, mc):
            for nb in range(NB):
                ps = psum.tile([128, NBW], f32, tag="sc")
                for kc in range(KC):
                    nc.tensor.matmul(
                        ps[:],
                        w_tiles[kc][:, ts(mc, 128)],
                        x_sb[kc][:, ts(nb, NBW)],
                        start=(kc == 0),
                        stop=(kc == KC - 1),
                    )
                nc.vector.tensor_add(
                    dst[:, ts(nb, NBW)],
                    ps[:],
                    bias[:].to_broadcast((128, NBW)),
                )

        # v in natural [t, d] orientation, with an all-ones denominator
        # column appended per head: [128, 4*65].
        v2_sb = [vpool.tile([128, HPC * (HD + 1)], f16, tag=f"v2{j}", name=f"v2{j}") for j in range(TCH)]
        for jb in range(TCH):
            nc.vector.memset(v2_sb[jb][:, HD : HPC * (HD + 1) : HD + 1], 1.0)

        def project_v(jbs):
            for jb in jbs:
                ps = psum.tile([128, DPC], f32, tag="sc")
                for kc in range(KC):
                    nc.tensor.matmul(
                        ps[:],
                        x_sb[kc][:, ts(jb, 128)],
                        wv_sb[kc][:],
                        start=(kc == 0),
                        stop=False,
                    )
                nc.tensor.matmul(  # + ones(t) x bias_v
                    ps[:],
                    ones_1xP[:],
                    bv_row[:],
                    start=False,
                    stop=True,
                )
                # copy all 4 head slices in one strided 3-D op
                v2 = v2_sb[jb]
                nc.vector.tensor_copy(
                    v2[:].rearrange("p (h c) -> p h c", h=HPC)[:, :, 0:HD],
                    ps[:].rearrange("p (h c) -> p h c", h=HPC),
                )

        # ---------------- attention for one head pair ----------------
        def attention_ib(pr, ib):
            qT, kT = qT_sb[pr], kT_sb[pr]
            nch = 4 * ib + 4  # causal key chunks per head
            # chunk stream: (hl, jb, qoff, width, d); d = diagonal
            # offset; d in {2,3} chunks cropped to the upper 256 cols.
            fulls, crops = [], []
            for jb in range(nch):
                d = jb - 4 * ib
                if d >= 2:
                    for hl in (0, 1):
                        crops.append((hl, jb, 256, 256, d))
                else:
                    for hl in (0, 1):
                        fulls.append((hl, jb, 0, 512, d))
            # groups: list of (chunk, col_off). Fulls: one (h, h') jb
            # pair per [128, 1024] group, cols 0/512. Crops: all four
            # 256-wide diagonal chunks in one group, laid out so
            # concurrently-issued (h, h') row-packed pairs never write
            # the same psum bank: h d2 -> 0, h' d2 -> 512, h d3 -> 256,
            # h' d3 -> 768.
            groups = []
            for i in range(0, len(fulls), 2):
                groups.append([(c, j * 512) for j, c in enumerate(fulls[i : i + 2])])
            if crops:
                cg = [
                    (crops[0], 0),
                    (crops[1], 512),
                    (crops[2], 256),
                    (crops[3], 768),
                ]
                groups.append(cg)

            po = [
                popool.tile([65, 512], f32, tag=f"po{hl}", name=f"po{pr}_{ib}_{hl}")
                for hl in (0, 1)
            ]
            for g in groups:
                width = sum(c[3] for c, _ in g)
                ps = psum.tile([128, width], f32, tag="sc")
                for (hl, jb, qo, w, d), off in g:
                    nc.tensor.matmul(
                        ps[:, off : off + w],
                        kT[ts(hl, 64), ts(jb, 128)],
                        qT[ts(hl, 64), ib * 512 + qo : ib * 512 + qo + w],
                        start=True,
                        stop=True,
                    )
                pT = ppool.tile([128, width], f16, tag="pT")
                # exp(scale*s + mask_bias); per-partition bias = additive
                # attention mask for this group's key chunk.
                if len(g) == 2:
                    # full group: both halves are the same key chunk jb
                    jb0 = g[0][0][1]
                    nc.scalar.activation(
                        pT[:], ps[:], EXP, scale=SM_SCALE,
                        bias=am_sb[:, jb0 : jb0 + 1],
                    )
                else:
                    # crop group: cols {0-255, 512-767} are chunk jb_a,
                    # cols {256-511, 768-1023} are chunk jb_b.
                    for ci, base in ((0, 0), (2, 256)):
                        jbx = g[ci][0][1]
                        nc.scalar.activation(
                            pT[:].rearrange("p (q h) -> p q h", q=2)[
                                :, :, base : base + 256
                            ],
                            ps[:].rearrange("p (q h) -> p q h", q=2)[
                                :, :, base : base + 256
                            ],
                            EXP,
                            scale=SM_SCALE,
                            bias=am_sb[:, jbx : jbx + 1],
                        )
                # causal staircase on partial chunks; merge the
                # (h, h') twin segments into one 3-D op when adjacent
                i = 0
                while i < len(g):
                    (hl, jb, qo, w, d), off_i = g[i]
                    if d < 0:
                        i += 1
                        continue
                    msk = stair_slice(d, w)
                    twin = (
                        i + 1 < len(g)
                        and g[i + 1][0][1] == jb
                        and g[i + 1][0][3] == w
                        and g[i + 1][1] == off_i + w
                    )
                    if twin:
                        seg = pT[:, off_i : off_i + 2 * w].rearrange(
                            "p (two n) -> p two n", two=2
                        )
                        nc.vector.tensor_mul(
                            seg,
                            seg,
                            msk.unsqueeze(1).broadcast_to((128, 2, w)),
                        )
                        i += 2
                    else:
                        nc.vector.tensor_mul(
                            pT[:, off_i : off_i + w],
                            pT[:, off_i : off_i + w],
                            msk,
                        )
                        i += 1
                # PV: outT[d, i] accumulation per head; denominator
                # (all-ones) column rides along as lhsT column 64.
                for (hl, jb, qo, w, d), off in g:
                    nc.tensor.matmul(
                        po[hl][:, qo : qo + w],
                        v2_sb[jb][:, (2 * pr + hl) * (HD + 1) : (2 * pr + hl + 1) * (HD + 1)],
                        pT[:, off : off + w],
                        start=(jb == 0),
                        stop=(jb == nch - 1),
                    )
            # normalize: out[:64] / denom (row 64), per column
            for hl in (0, 1):
                sbp = opool.tile([65, 512], f32, tag="sbp")
                nc.vector.tensor_copy(sbp[:], po[hl][:])
                # reshape the denominator row onto 128 partitions so the
                # (slow, free-size-bound) DVE reciprocal runs on [128, 4]
                dn = opool.tile([128, 4], f32, tag="dn")
                nc.gpsimd.dma_start(
                    dn[:], sbp[64:65, :].rearrange("o (p c) -> o p c", p=128)
                )
                dnr = opool.tile([128, 4], f32, tag="dnr")
                nc.vector.reciprocal(dnr[:], dn[:])
                rc = opool.tile([1, 512], f32, tag="rc")
                nc.sync.dma_start(
                    rc[:].rearrange("o (p c) -> o p c", p=128), dnr[:]
                )
                rb = opool.tile([64, 512], f32, tag="rb")
                nc.gpsimd.partition_broadcast(rb[:], rc[:])
                oT = opool.tile([64, 512], f32, tag="oT")
                nc.vector.tensor_mul(oT[:], sbp[0:64, :], rb[:])
                nc.sync.dma_start(
                    outT[(2 * pr + hl) * HD : (2 * pr + hl + 1) * HD, ts(ib, 512)],
                    oT[:],
                )

        # emission order chosen for overlap: pair-0 q/k first, then v,
        # then pair-0 attention with pair-1 projections interleaved as
        # PE filler work (keeps PE dense/warm while ACT exps pair 0).
        project_qk(wq_sb, qT_sb[0], bias_q[0], 0)
        project_qk(wk_sb, kT_sb[0], bias_k[0], 0)
        project_v(range(0, 4))
        attention_ib(0, 0)
        project_v(range(4, 8))
        attention_ib(0, 1)
        project_qk(wq_sb, qT_sb[1], bias_q[1], 1)
        project_v(range(8, 12))
        attention_ib(0, 2)
        project_qk(wk_sb, kT_sb[1], bias_k[1], 1)
        project_v(range(12, 16))
        attention_ib(0, 3)
        for ib in range(IBN):
            attention_ib(1, ib)

    nc.compile()
    return nc


def _shard_inputs(inputs):
    """Full inputs -> per-core input maps (host-side layout + fp16 cast)."""
    f16 = np.float16
    hs = np.asarray(inputs["hidden_states"], dtype=np.float32)
    am = np.asarray(inputs["attention_mask"], dtype=np.float32)
    Wq = np.asarray(inputs["Wq"], dtype=np.float32)
    Wk = np.asarray(inputs["Wk"], dtype=np.float32)
    Wv = np.asarray(inputs["Wv"], dtype=np.float32)
    bq = np.asarray(inputs["bq"], dtype=np.float32)
    bk = np.asarray(inputs["bk"], dtype=np.float32)
    bv = np.asarray(inputs["bv"], dtype=np.float32)
    Aq = np.asarray(inputs["Aq"], dtype=np.float32)
    Bq = np.asarray(inputs["Bq"], dtype=np.float32)
    Av = np.asarray(inputs["Av"], dtype=np.float32)
    Bv = np.asarray(inputs["Bv"], dtype=np.float32)

    c = np.ascontiguousarray
    xTs = [c(hs[b].T.astype(f16)) for b in range(B)]
    aq16 = c(Aq.astype(f16))
    av16 = c(Av.astype(f16))
    in_maps = []
    for core in range(NCORES):
        b, g = core // GPB, core % GPB
        sl = slice(g * DPC, (g + 1) * DPC)
        in_maps.append(
            {
                "xT": xTs[b],
                "wqT": c(Wq[sl].T.astype(f16)),
                "wkT": c(Wk[sl].T.astype(f16)),
                "wvT": c(Wv[sl].T.astype(f16)),
                "aq": aq16,
                "av": av16,
                "bqT": c((LORA_SCALE * Bq[sl].T).astype(f16)),
                "bvT": c((LORA_SCALE * Bv[sl].T).astype(f16)),
                "biasq": c(bq[sl]),
                "biask": c(bk[sl]),
                "biasv": c(bv[sl].astype(f16)),
                "amask": c(am[b, 0, 0, :]),
            }
        )
    return in_maps


def _run(inputs, trace=False):
    from concourse.bass_utils import run_bass_kernel_spmd

    if "nc" not in _CACHE:
        _CACHE["nc"] = _build_program()
    nc = _CACHE["nc"]
    in_maps = _shard_inputs(inputs)
    res = run_bass_kernel_spmd(nc, in_maps, list(range(NCORES)), trace=trace)
    out = np.empty((B, T, DM), dtype=np.float32)
    for core in range(NCORES):
        b, g = core // GPB, core % GPB
        out[b, :, g * DPC : (g + 1) * DPC] = res.results[core]["outT"].T
    return out, res


def kernel(**inputs) -> np.ndarray:
    out, _ = _run(inputs, trace=False)
    return out


# revision 5
# speedup vs baseline: 1.3658x; 1.0303x over previous
"""Causal self-attention with LoRA (q,v) — Trainium2 Bass kernel, 8 cores.

Sharding: data-parallel over batch (B=2), tensor-parallel over heads
(16 heads -> 4 per core).  Core c handles batch c//4, heads 4*(c%4)..+4.
Each core computes its 256-dim q/k/v projection slice from the full
hidden states and its heads' full 2048x2048 causal attention locally.
No collectives; host does the (layout-only) scatter/gather + fp16 casts
+ sbuf-image tiling so every bulk load is one contiguous HWDGE DMA.

The whole datapath runs in fp16 with fp32 PSUM accumulation (measured
rel err vs the fp32 reference ~5e-4).  fp16 matters a lot on TRN2: f32r
matmuls cannot use standalone LDWEIGHTS (walrus bug) so every f32r MM
pays an inline ~214ns weight load, and FP32_HIGH disables fast weight
load entirely; fp16 streams at 1 col/cycle with LDW hidden by the PE
reorder window.

Attention is computed in transposed orientation (scores sT[j, i]) so no
on-chip transposes are needed; the softmax denominator rides along the
PV matmul as a 65th lhsT column (augmented-V), which also folds the
additive attention mask in exactly (em = exp(mask) scaling of V rows).
Score psum groups pack 3 head-chunks (1536 cols) so the ACT exp — the
throughput limiter of the attention phase at 1 elem/lane/cycle — pays
its ~352-cycle per-instruction overhead a third as often.  Causal
masking is a multiplicative staircase applied after exp; the two
most-diagonal 128-key chunks per 512-query block are cropped to their
upper 256 columns (laid out so concurrently issued row-packed matmul
pairs never write the same PSUM bank).  Scores for the two heads of a
pair run concurrently via K=64 row packing.  Emission follows x-quarter
arrival so the first attention block starts ~10us in.
"""

import numpy as np

B, T, DM, H = 2, 2048, 1024, 16
HD = 64          # head dim
R = 8            # LoRA rank
NCORES = 8
GPB = 4          # head-groups (cores) per batch
HPC = 4          # heads per core
DPC = HPC * HD   # 256 output dims per core
LORA_SCALE = 2.0  # lora_alpha / r
SM_SCALE = HD ** -0.5  # 0.125

KC = DM // 128   # 8 contraction chunks
MC = DPC // 128  # 2 output-dim chunks (head pairs)
NB = 4           # t-blocks (x quarters) for q/k projections
TCH = T // 128   # 16 t-chunks (key chunks)
IBN = T // 512   # 4 query i-blocks (512 wide)
GCH = 3          # score-group capacity in 512-wide chunk units

_CACHE = {}


def _build_program():
    from contextlib import ExitStack

    import concourse.bass as bass
    import concourse.tile as tile
    from concourse import bacc, mybir

    f32 = mybir.dt.float32
    f16 = mybir.dt.float16
    EXP = mybir.ActivationFunctionType.Exp
    ts = bass.ts

    nc = bacc.Bacc(
        "TRN2",
        target_bir_lowering=False,
        debug=False,
        enable_asserts=True,
        num_devices=NCORES,
    )

    xq = nc.dram_tensor("xq", [NB, 128, KC * 512], f16, kind="ExternalInput").ap()
    wq_img = nc.dram_tensor("wq_img", [128, KC * DPC], f16, kind="ExternalInput").ap()
    wk_img = nc.dram_tensor("wk_img", [128, KC * DPC], f16, kind="ExternalInput").ap()
    wv_img = nc.dram_tensor("wv_img", [128, KC * DPC], f16, kind="ExternalInput").ap()
    a_both = nc.dram_tensor("a_both", [R, 2 * DM], f16, kind="ExternalInput").ap()
    bT_both = nc.dram_tensor("bT_both", [R, 2 * DPC], f16, kind="ExternalInput").ap()
    biasqk = nc.dram_tensor("biasqk", [128, 4], f32, kind="ExternalInput").ap()
    biasv = nc.dram_tensor("biasv", [DPC], f16, kind="ExternalInput").ap()
    amask = nc.dram_tensor("amask", [T], f32, kind="ExternalInput").ap()
    outT = nc.dram_tensor("outT", [DPC, T], f32, kind="ExternalOutput").ap()

    with tile.TileContext(nc) as tc, ExitStack() as ctx:
        const = ctx.enter_context(tc.tile_pool(name="const", bufs=1))
        xpool = ctx.enter_context(tc.tile_pool(name="x", bufs=1))
        wpool = ctx.enter_context(tc.tile_pool(name="w", bufs=1))
        qkpool = ctx.enter_context(tc.tile_pool(name="qk", bufs=1))
        vpool = ctx.enter_context(tc.tile_pool(name="v", bufs=1))
        ppool = ctx.enter_context(tc.tile_pool(name="pT", bufs=4))
        opool = ctx.enter_context(tc.tile_pool(name="osb", bufs=4))
        psum = ctx.enter_context(tc.tile_pool(name="psum", bufs=2, space="PSUM"))
        popool = ctx.enter_context(tc.tile_pool(name="po", bufs=1, space="PSUM"))

        def stair_slice(d, w):
            # full-width chunk (w=512): mask[p, f] = f >= p + 128 d
            # cropped chunk  (w=256, f' = f-256): f' >= p + 128 d - 256
            start = 384 - 128 * d if w == 512 else 640 - 128 * d
            return stair[:, start : start + w]

        # ---------------- constant loads (sync queue) ----------------
        bias_sb = const.tile([128, 4], f32, tag="biasqk")
        nc.sync.dma_start(bias_sb[:], biasqk)
        bv_row = const.tile([1, DPC], f16, tag="bvrow")
        nc.sync.dma_start(bv_row[:], biasv.unsqueeze(0))
        em_raw = const.tile([128, TCH], f32, tag="em_raw")
        nc.sync.dma_start(em_raw[:], amask.rearrange("(c p) -> p c", p=128))
        a_sb = const.tile([R, 2 * DM], f16, tag="a")
        nc.sync.dma_start(a_sb[:], a_both)
        bT_sb = const.tile([R, 2 * DPC], f16, tag="bT")
        nc.sync.dma_start(bT_sb[:], bT_both)

        # ---------------- x: 4 one-shot quarter DMAs (scalar queue) --
        xall = xpool.tile([128, KC * T], f16, tag="xall")
        x3d = xall[:].rearrange("p (kc c) -> p kc c", kc=KC)
        for q in range(NB):
            nc.scalar.dma_start(x3d[:, :, ts(q, 512)], xq[q])

        def xs(kc, lo, n):
            return xall[:, 2048 * kc + lo : 2048 * kc + lo + n]

        # ---------------- weights: one DMA per matrix (sync) ---------
        wq_all = wpool.tile([128, KC * DPC], f16, tag="wq_all")
        nc.sync.dma_start(wq_all[:], wq_img)
        wk_all = wpool.tile([128, KC * DPC], f16, tag="wk_all")
        nc.sync.dma_start(wk_all[:], wk_img)
        wv_all = wpool.tile([128, KC * DPC], f16, tag="wv_all")
        nc.sync.dma_start(wv_all[:], wv_img)

        ones_1xP = const.tile([1, 128], f16, tag="ones")
        nc.vector.memset(ones_1xP[:], 1.0)

        # em[p, jb] = exp(amask[128*jb + p])
        em = const.tile([128, TCH], f32, tag="em")
        nc.scalar.activation(em[:], em_raw[:], EXP)

        # Causal staircase (multiplicative, applied after exp):
        # stair[p, m] = 1.0 if m >= p + 384 else 0.0 ; shape [128, 896].
        stair = const.tile([128, 896], f16, tag="stair")
        nc.gpsimd.memset(stair[:], 1.0)
        nc.gpsimd.affine_select(
            out=stair[:],
            in_=stair[:],
            compare_op=mybir.AluOpType.is_ge,
            fill=0.0,
            base=-384,
            pattern=[[1, 896]],
            channel_multiplier=-1,
        )

        # ---------------- weights with LoRA fold for q, v ---------------
        def load_folded(w_all, a_off, bT_off, name):
            """W'.T = W.T + A.T @ (2 B.T), one [128, KC*DPC] tile."""
            wf = wpool.tile([128, KC * DPC], f16, tag=f"wf_{name}")
            for kc in range(KC):
                dps = psum.tile([128, DPC], f32, tag="sc")
                nc.tensor.matmul(
                    dps[:],
                    a_sb[:, a_off + 128 * kc : a_off + 128 * kc + 128],
                    bT_sb[:, bT_off : bT_off + DPC],
                    start=True,
                    stop=True,
                )
                nc.vector.tensor_add(
                    wf[:, ts(kc, DPC)], w_all[:, ts(kc, DPC)], dps[:]
                )
            return wf

        wq_f = load_folded(wq_all, 0, 0, "q")
        wv_f = load_folded(wv_all, DM, DPC, "v")

        # ---------------- projections ----------------
        # qT/kT: [d, t] with d on partitions; tile mc holds head pair
        # (2mc, 2mc+1): partitions 0-63 = head 2mc, 64-127 = head 2mc+1.
        qT_sb = [qkpool.tile([128, T], f16, tag=f"qT{mc}", name=f"qT{mc}") for mc in range(MC)]
        kT_sb = [qkpool.tile([128, T], f16, tag=f"kT{mc}", name=f"kT{mc}") for mc in range(MC)]

        def project_qk(wf, dst, bias, mc, nb):
            ps = psum.tile([128, 512], f32, tag="sc")
            for kc in range(KC):
                nc.tensor.matmul(
                    ps[:],
                    wf[:, kc * DPC + mc * 128 : kc * DPC + mc * 128 + 128],
                    xs(kc, 512 * nb, 512),
                    start=(kc == 0),
                    stop=(kc == KC - 1),
                )
            nc.vector.tensor_add(
                dst[:, ts(nb, 512)],
                ps[:],
                bias.to_broadcast((128, 512)),
            )

        # v in natural [t, d] orientation, em-scaled, with the denominator
        # (em) column appended per head: [128, 4*65].
        v2_sb = [vpool.tile([128, HPC * (HD + 1)], f16, tag=f"v2{j}", name=f"v2{j}") for j in range(TCH)]

        def project_v(jbs):
            for jb in jbs:
                ps = psum.tile([128, DPC], f32, tag="sc")
                for kc in range(KC):
                    nc.tensor.matmul(
                        ps[:],
                        xs(kc, 128 * jb, 128),
                        wv_f[:, ts(kc, DPC)],
                        start=(kc == 0),
                        stop=False,
                    )
                nc.tensor.matmul(  # + ones(t) x bias_v
                    ps[:],
                    ones_1xP[:],
                    bv_row[:],
                    start=False,
                    stop=True,
                )
                v2 = v2_sb[jb]
                em_col = em[:, jb : jb + 1]
                nc.vector.tensor_mul(
                    v2[:].rearrange("p (h c) -> p h c", h=HPC)[:, :, 0:HD],
                    ps[:].rearrange("p (h c) -> p h c", h=HPC),
                    em_col.unsqueeze(1).broadcast_to((128, HPC, HD)),
                )
                nc.vector.tensor_copy(
                    v2[:, HD : HPC * (HD + 1) : HD + 1],
                    em_col.to_broadcast((128, HPC)),
                )

        # ---------------- attention for one head pair ----------------
        def attention_ib(pr, ib):
            qT, kT = qT_sb[pr], kT_sb[pr]
            nch = 4 * ib + 4  # causal key chunks per head
            # chunk stream: (hl, jb, qoff, width, d); d = diagonal
            # offset; d in {2,3} chunks cropped to the upper 256 cols.
            fulls, crops = [], []
            for jb in range(nch):
                d = jb - 4 * ib
                if d >= 2:
                    for hl in (0, 1):
                        crops.append((hl, jb, 256, 256, d))
                else:
                    for hl in (0, 1):
                        fulls.append((hl, jb, 0, 512, d))
            # groups: list of (chunk, col_off).  Fulls are packed GCH
            # chunks to a psum group (column offsets 0/512/1024) so one
            # ACT exp instruction covers up to 1536 columns.  Adjacent
            # offsets always land in different psum banks, so the
            # concurrently-running row-packed (h, h') score pairs never
            # write the same bank.  Crops: all four 256-wide diagonal
            # chunks in one group: h d2 -> 0, h' d2 -> 512, h d3 -> 256,
            # h' d3 -> 768.
            groups = []
            for i in range(0, len(fulls), GCH):
                groups.append([(c, j * 512) for j, c in enumerate(fulls[i : i + GCH])])
            if crops:
                cg = [
                    (crops[0], 0),
                    (crops[1], 512),
                    (crops[2], 256),
                    (crops[3], 768),
                ]
                groups.append(cg)

            po = [
                popool.tile([65, 512], f32, tag=f"po{hl}", name=f"po{pr}_{ib}_{hl}")
                for hl in (0, 1)
            ]
            for g in groups:
                width = sum(c[3] for c, _ in g)
                ps = psum.tile([128, width], f32, tag="sc")
                for (hl, jb, qo, w, d), off in g:
                    nc.tensor.matmul(
                        ps[:, off : off + w],
                        kT[ts(hl, 64), ts(jb, 128)],
                        qT[ts(hl, 64), ib * 512 + qo : ib * 512 + qo + w],
                        start=True,
                        stop=True,
                    )
                pT = ppool.tile([128, width], f16, tag="pT")
                nc.scalar.activation(pT[:], ps[:], EXP, scale=SM_SCALE)
                # causal staircase on partial chunks; merge the
                # (h, h') twin segments into one 3-D op when adjacent
                i = 0
                while i < len(g):
                    (hl, jb, qo, w, d), off_i = g[i]
                    if d < 0:
                        i += 1
                        continue
                    msk = stair_slice(d, w)
                    twin = (
                        i + 1 < len(g)
                        and g[i + 1][0][1] == jb
                        and g[i + 1][0][3] == w
                        and g[i + 1][1] == off_i + w
                    )
                    if twin:
                        seg = pT[:, off_i : off_i + 2 * w].rearrange(
                            "p (two n) -> p two n", two=2
                        )
                        nc.vector.tensor_mul(
                            seg,
                            seg,
                            msk.unsqueeze(1).broadcast_to((128, 2, w)),
                        )
                        i += 2
                    else:
                        nc.vector.tensor_mul(
                            pT[:, off_i : off_i + w],
                            pT[:, off_i : off_i + w],
                            msk,
                        )
                        i += 1
                # PV: outT[d, i] accumulation per head; denominator
                # column (em) rides along as lhsT column 64.
                for (hl, jb, qo, w, d), off in g:
                    nc.tensor.matmul(
                        po[hl][:, qo : qo + w],
                        v2_sb[jb][:, (2 * pr + hl) * (HD + 1) : (2 * pr + hl + 1) * (HD + 1)],
                        pT[:, off : off + w],
                        start=(jb == 0),
                        stop=(jb == nch - 1),
                    )
            # normalize: out[:64] / denom (row 64), per column
            for hl in (0, 1):
                sbp = opool.tile([65, 512], f32, tag="sbp")
                nc.vector.tensor_copy(sbp[:], po[hl][:])
                # reshape the denominator row onto 128 partitions so the
                # (slow, free-size-bound) DVE reciprocal runs on [128, 4]
                dn = opool.tile([128, 4], f32, tag="dn")
                nc.gpsimd.dma_start(
                    dn[:], sbp[64:65, :].rearrange("o (p c) -> o p c", p=128)
                )
                dnr = opool.tile([128, 4], f32, tag="dnr")
                nc.vector.reciprocal(dnr[:], dn[:])
                rc = opool.tile([1, 512], f32, tag="rc")
                nc.sync.dma_start(
                    rc[:].rearrange("o (p c) -> o p c", p=128), dnr[:]
                )
                rb = opool.tile([64, 512], f32, tag="rb")
                nc.gpsimd.partition_broadcast(rb[:], rc[:])
                oT = opool.tile([64, 512], f32, tag="oT")
                nc.vector.tensor_mul(oT[:], sbp[0:64, :], rb[:])
                nc.sync.dma_start(
                    outT[(2 * pr + hl) * HD : (2 * pr + hl + 1) * HD, ts(ib, 512)],
                    oT[:],
                )

        # emission order follows x-quarter arrival: pair-0 projections
        # and attention per quarter, then pair-1 projections interleaved
        # with pair-1 attention as PE filler while ACT exps.
        for nb in range(NB):
            project_qk(wq_f, qT_sb[0], bias_sb[:, 0:1], 0, nb)
            project_qk(wk_all, kT_sb[0], bias_sb[:, 2:3], 0, nb)
            project_v(range(4 * nb, 4 * nb + 4))
            attention_ib(0, nb)
        for nb in range(NB):
            project_qk(wq_f, qT_sb[1], bias_sb[:, 1:2], 1, nb)
            project_qk(wk_all, kT_sb[1], bias_sb[:, 3:4], 1, nb)
            attention_ib(1, nb)

    nc.compile()
    return nc


def _shard_inputs(inputs):
    """Full inputs -> per-core input maps (host-side layout + fp16 cast)."""
    f16 = np.float16
    hs = np.asarray(inputs["hidden_states"], dtype=np.float32)
    am = np.asarray(inputs["attention_mask"], dtype=np.float32)
    Wq = np.asarray(inputs["Wq"], dtype=np.float32)
    Wk = np.asarray(inputs["Wk"], dtype=np.float32)
    Wv = np.asarray(inputs["Wv"], dtype=np.float32)
    bq = np.asarray(inputs["bq"], dtype=np.float32)
    bk = np.asarray(inputs["bk"], dtype=np.float32)
    bv = np.asarray(inputs["bv"], dtype=np.float32)
    Aq = np.asarray(inputs["Aq"], dtype=np.float32)
    Bq = np.asarray(inputs["Bq"], dtype=np.float32)
    Av = np.asarray(inputs["Av"], dtype=np.float32)
    Bv = np.asarray(inputs["Bv"], dtype=np.float32)

    c = np.ascontiguousarray

    def wimg(W, sl):
        # sbuf image: wimg[p, 256*kc + j] = W[sl].T[128*kc + p, j]
        return c(W[sl].T.astype(f16).reshape(KC, 128, DPC).transpose(1, 0, 2).reshape(128, KC * DPC))

    # x quarter images: Xq[q, p, 512*kc + cc] = hs[b].T[128*kc + p, 512*q + cc]
    xqs = [
        c(hs[b].T.astype(f16).reshape(KC, 128, NB, 512).transpose(2, 1, 0, 3).reshape(NB, 128, KC * 512))
        for b in range(B)
    ]
    a_both = c(np.concatenate([Aq, Av], axis=1).astype(f16))
    in_maps = []
    for core in range(NCORES):
        b, g = core // GPB, core % GPB
        sl = slice(g * DPC, (g + 1) * DPC)
        bqk = np.stack(
            [bq[sl][:128], bq[sl][128:], bk[sl][:128], bk[sl][128:]], axis=1
        )
        bT = np.concatenate(
            [LORA_SCALE * Bq[sl].T, LORA_SCALE * Bv[sl].T], axis=1
        ).astype(f16)
        in_maps.append(
            {
                "xq": xqs[b],
                "wq_img": wimg(Wq, sl),
                "wk_img": wimg(Wk, sl),
                "wv_img": wimg(Wv, sl),
                "a_both": a_both,
                "bT_both": c(bT),
                "biasqk": c(bqk),
                "biasv": c(bv[sl].astype(f16)),
                "amask": c(am[b, 0, 0, :]),
            }
        )
    return in_maps


def _run(inputs, trace=False):
    from concourse.bass_utils import run_bass_kernel_spmd

    if "nc" not in _CACHE:
        _CACHE["nc"] = _build_program()
    nc = _CACHE["nc"]
    in_maps = _shard_inputs(inputs)
    res = run_bass_kernel_spmd(nc, in_maps, list(range(NCORES)), trace=trace)
    out = np.empty((B, T, DM), dtype=np.float32)
    for core in range(NCORES):
        b, g = core // GPB, core % GPB
        out[b, :, g * DPC : (g + 1) * DPC] = res.results[core]["outT"].T
    return out, res


def kernel(**inputs) -> np.ndarray:
    out, _ = _run(inputs, trace=False)
    return out
